# revision 22
# baseline (speedup 1.0000x reference)
"""CFConv Trainium2 kernel.

Math: out[b,o,y,x] = sum_{k,i,j} weight[k,o,i,j] * fa[b,i,y+dy,x+dx] * wa[b,j,y+dy,x+dx]
(3x3 valid conv over the outer-product channel space of fa (65ch) x wa (17ch)).

Strategy (8 NeuronCores, SPMD):
- Shard (batch b, row-half h): each core computes 63 output rows of one batch.
- On-chip, form z[(i,j), pix] = f_i * w_j for the 64x16 "main" (i,j) grid as
  8 partition-chunks of 128 (j-minor), by an elementwise multiply of a
  host-pre-replicated f image (frep, streamed from HBM per pixel window)
  against a pre-tiled copy of w. The remaining 81 channels (j=16 ones
  column, i=64 ones row, corner) are read directly from a packed
  [f; w; ones] tensor.
- Contract with the tensor engine in fp16 (fp32 PSUM accumulation). Matmuls
  are issued in column-tiled pairs (tile_position (0,0)/(0,64)): the two
  64-wide PE column groups concurrently compute two adjacent pixel-tile
  outputs, accumulating into the lower/upper partition halves of one PSUM
  bank. Pairs 0-6 use N=512 (4 output rows per group); the last pair uses
  N=448 covering exactly the remaining 7 rows.
- Output layout stays at input width (128) so all 9 conv offsets are plain
  column shifts; the two garbage columns per row are skipped at DMA-out.
"""

import numpy as np

B, WCH, FCH, OCH, H, W = 4, 16, 64, 64, 128, 128
KX = 3
HO = WO = H - KX + 1          # 126
ROWS_OUT = 63                 # output rows per core
ROWS_IN = 65                  # input rows per core
FREE = 8448                   # padded region width (66 rows * 128)
VALID = ROWS_IN * W           # 8320
NPAIR = 8                     # pixel-tile pairs per core
HALO = 2 * W + 2              # 258

_cache = {}


def _pair_geom(a):
    # pairs 0-6: N=512 (rows 8a..8a+8); pair 7: N=448 (rows 56..63)
    if a < NPAIR - 1:
        return 1024 * a, 512
    return 7168, 448


def _build_program():
    import concourse.bacc as bacc
    import concourse.mybir as mybir
    import concourse.tile as tile

    f16 = mybir.dt.float16
    f32 = mybir.dt.float32

    nc = bacc.Bacc("TRN2", target_bir_lowering=False, enable_partition_id=False)
    # The 47 zero rows that pad the prelude K-chunk to 128 (HAM needs
    # K=128 to stay armed) sit at partitions 0:47. Only the critical
    # enabler piece (cols 0:1344) ships the 81 live rows with the pad
    # memset on-chip — 37% off the first DMA; later pieces ship the
    # zeros too, since DMA has slack there and on-chip memsets measured
    # slower than the stream (DVE memsets serialize at ~2.5us/piece and
    # stalled the prelude into a HAM re-throttle).
    fw_d = nc.dram_tensor("fw", (128, FREE), f16, kind="ExternalInput")
    wkx_d = nc.dram_tensor("wkx", (81, 9 * 64), f16, kind="ExternalInput")
    frep_d = nc.dram_tensor("frep", (128, 8 * FREE), f16, kind="ExternalInput")
    wt_d = nc.dram_tensor("wt", (128, FREE), f16, kind="ExternalInput")
    wkm_d = nc.dram_tensor("wkm", (128, 9 * 8 * 64), f16, kind="ExternalInput")
    # full input width (128) so each 4-row output block is one contiguous
    # 2KB-per-partition DMA run; the host slices off the 2 garbage columns.
    out_d = nc.dram_tensor("out", (OCH, ROWS_OUT, W), f32, kind="ExternalOutput")

    with tile.TileContext(nc) as tc:
        with tc.tile_pool(name="inp", bufs=1) as inp, \
             tc.tile_pool(name="f7", bufs=1) as f7p, \
             tc.tile_pool(name="frep", bufs=2) as freps, \
             tc.tile_pool(name="z", bufs=2) as zp, \
             tc.tile_pool(name="st", bufs=2) as stp, \
             tc.tile_pool(name="ps", bufs=1, space="PSUM") as psp:
            # All 8 PSUM banks stay resident, one per pixel-tile pair: a
            # PRELUDE runs the fw-chunk (c=8) matmuls of every pair first.
            # It needs only ~0.36MB of DMA (wkx + the 81-row head of fw),
            # giving the 21MB frep stream a ~15us head start before the
            # first z-chunk matmul. fw/wkx are zero-padded to K=128: K<128
            # matmuls do not arm the HAM activity detector, and a K=81
            # prelude would run at the cold 1.2GHz clock throughout.
            ps_tiles = [psp.tile([128, 512], f32, tag=f"ps{a}", name=f"ps{a}")
                        for a in range(NPAIR)]

            # dummy matmuls warm the PE clock (HAM) while fw lands; fed by
            # a gpsimd memset (no DMA dependency). They write into pair
            # 7's bank, which that pair's start=True matmul later resets.
            # 26 dummies (~4.4µs mixed cold/warm) bridge first-mm (~7µs)
            # to the fw_p0 DMA sem (~11.5-14µs) while arming the HAM
            # window. Any tensor-idle gap >~3.4µs here lets HAM
            # re-throttle, making the prelude run at 1.2GHz (measured:
            # costs more than the gap itself). 18 was not enough to arm
            # reliably; 22+ is.
            warm = inp.tile([128, 256], f16)
            nc.gpsimd.memset(warm[:], 0.0)
            for _ in range(26):
                nc.tensor.matmul(ps_tiles[7][0:64, 0:256], warm[:, 0:64], warm[:, 0:256],
                                 start=True, stop=True, tile_position=(0, 0))

            fw_s = inp.tile([128, FREE], f16)
            wt_s = inp.tile([128, FREE], f16)
            wkm_s = inp.tile([128, 9 * 8 * 64], f16)
            wkx_s = inp.tile([128, 9 * 64], f16)

            # DMA rings, in consumption order. Measured: early DMA runs
            # at only ~125-200 GB/s (all 8 cores stream HBM at once) and
            # each link's completion sem lags its last byte by ~1.4us.
            # The prelude enabler is fw_p0 alone on the sync ring (219KB,
            # 81 live rows; sem ~11.5-12.5us); wkx rides the scalar ring
            # concurrently so the two sem lags overlap. Later fw pieces
            # land a pair ahead of prelude consumption (1.92us/pair);
            # wt/wkm/the split big0 follow, ahead of the main loop's
            # first z-chunk matmul (~27us).
            first = 1344
            # zero the K-pad rows 0:47 on the (idle) vector engine; the
            # wkx pad must be exactly 0 and the fw pad merely finite
            # (0 * NaN would poison PSUM), so both are zeroed.
            nc.vector.memset(wkx_s[0:47, :], 0.0)
            nc.vector.memset(fw_s[0:47, 0:1344], 0.0)
            nc.scalar.dma_start(wkx_s[47:128, :], wkx_d[:, :])
            nc.sync.dma_start(fw_s[47:128, 0:1344], fw_d[47:128, 0:1344])
            nc.sync.dma_start(fw_s[:, 1344:2368], fw_d[:, 1344:2368])
            nc.sync.dma_start(fw_s[:, 2368:4480], fw_d[:, 2368:4480])
            nc.sync.dma_start(fw_s[:, 4480:6592], fw_d[:, 4480:6592])
            nc.sync.dma_start(fw_s[:, 6592:FREE], fw_d[:, 6592:FREE])
            nc.sync.dma_start(wt_s[:, 0:first], wt_d[:, 0:first])
            frep_view = frep_d[:, :].rearrange("p (c f) -> p c f", f=FREE)
            win0 = 2 * 512 + HALO
            big0 = freps.tile([128, 8 * win0], f16, tag="frep", name="big0")
            big0_v = big0[:].rearrange("p (c f) -> p c f", f=win0)
            nc.sync.dma_start(big0_v[:, 0:4, :], frep_view[:, 0:4, 0:win0])
            nc.sync.dma_start(wkm_s[:, 0:2304], wkm_d[:, 0:2304])
            nc.sync.dma_start(big0_v[:, 4:8, :], frep_view[:, 4:8, 0:win0])
            nc.sync.dma_start(wkm_s[:, 2304:4608], wkm_d[:, 2304:4608])
            pw = (FREE - first) // 3
            def piece(eng, i):
                lo = first + i * pw
                hi = first + (i + 1) * pw if i < 2 else FREE
                eng.dma_start(wt_s[:, lo:hi], wt_d[:, lo:hi])
            piece(nc.scalar, 0)

            # prelude: c=8 (fw) matmuls for every pair
            for a in range(NPAIR):
                c0, n0 = _pair_geom(a)
                for k in range(9):
                    dy, dx = divmod(k, KX)
                    d = dy * W + dx
                    for g in (0, 1):
                        off = g * n0
                        nc.tensor.matmul(
                            ps_tiles[a][64 * g:64 * g + 64, 0:n0],
                            wkx_s[:, k * 64:k * 64 + 64],
                            fw_s[:, c0 + d + off:c0 + d + off + n0],
                            start=(k == 0), stop=False,
                            tile_position=(0, 64 * g),
                        )

            for a in range(NPAIR):
                c0, n0 = _pair_geom(a)
                win = 2 * n0 + HALO

                if a == 0:
                    big = big0
                else:
                    pool = freps if a < NPAIR - 1 else f7p
                    big = pool.tile([128, 8 * win], f16, name="big",
                                    tag="frep" if a < NPAIR - 1 else "frep7")
                    nc.sync.dma_start(
                        big[:].rearrange("p (c f) -> p c f", f=win),
                        frep_view[:, :, c0:c0 + win])
                zs = []
                for c in range(8):
                    z = zp.tile([128, win], f16, tag=f"z{c}", name=f"z{c}")
                    nc.vector.tensor_mul(z[:], big[:, c * win:c * win + win],
                                         wt_s[:, c0:c0 + win])
                    zs.append(z)
                if a < 2:
                    piece(nc.scalar, a + 1)

                ps = ps_tiles[a]
                for c in range(8):
                    for k in range(9):
                        dy, dx = divmod(k, KX)
                        d = dy * W + dx
                        for g in (0, 1):
                            off = g * n0
                            nc.tensor.matmul(
                                ps[64 * g:64 * g + 64, 0:n0],
                                wkm_s[:, (c * 9 + k) * 64:(c * 9 + k) * 64 + 64],
                                zs[c][:, d + off:d + off + n0],
                                start=False,
                                stop=(c == 7 and k == 8),
                                tile_position=(0, 64 * g),
                            )

                stage = stp.tile([128, 512], f32)
                if a < NPAIR - 1:
                    nc.vector.tensor_copy(stage[:, 0:n0], ps[:, 0:n0])
                    # one output block per ring, so neither ring is busy
                    # when the final pair's drain needs it
                    for g, eng in ((0, nc.sync), (1, nc.scalar)):
                        r_dst = 8 * a + 4 * g
                        src = stage[64 * g:64 * g + 64, 0:4 * W].rearrange(
                            "p (r c) -> p r c", c=W)
                        eng.dma_start(out_d[:, r_dst:r_dst + 4, :], src)
                else:
                    # drain tail: the two group halves copy in parallel
                    # (vector / scalar — the only engines with PSUM read
                    # ports), then DMA out on the two idle rings as flat
                    # contiguous [64, 448] runs (rows*W is contiguous
                    # per och partition, so the half-row split needs no
                    # special-casing).
                    out_flat = out_d[:, :, :].rearrange("o r c -> o (r c)")
                    nc.vector.tensor_copy(stage[0:64, 0:n0], ps[0:64, 0:n0])
                    nc.scalar.copy(stage[64:128, 0:n0], ps[64:128, 0:n0])
                    nc.sync.dma_start(out_flat[:, c0:c0 + n0],
                                      stage[0:64, 0:n0])
                    nc.scalar.dma_start(out_flat[:, c0 + n0:c0 + 2 * n0],
                                        stage[64:128, 0:n0])

    nc.finalize()
    return nc


def _prep_core(inputf, inputw, b, h):
    r0 = 63 * h
    f_reg = np.zeros((64, FREE), np.float16)
    f_reg[:, :VALID] = inputf[b, :, r0:r0 + ROWS_IN, :].reshape(64, VALID)
    w_reg = np.zeros((16, FREE), np.float16)
    w_reg[:, :VALID] = inputw[b, :, r0:r0 + ROWS_IN, :].reshape(16, VALID)
    ones_reg = np.zeros((1, FREE), np.float16)
    ones_reg[0, :VALID] = 1.0
    # K-pad rows at partitions 0:47; the enabler DMA piece skips them
    # (they are memset on-chip for cols 0:1344)
    pad_reg = np.zeros((47, FREE), np.float16)
    fw = np.concatenate([pad_reg, f_reg, w_reg, ones_reg], 0)

    # frep[p, c*FREE + col] = f_reg[8c + p//16, col]
    rows = (8 * np.arange(8)[None, :] + (np.arange(128) // 16)[:, None])  # [128, 8]
    frep = np.ascontiguousarray(f_reg[rows].reshape(128, 8 * FREE))

    wt = np.empty((128, FREE), np.float16)
    for u in range(8):
        wt[16 * u:16 * u + 16] = w_reg
    return fw, frep, wt


def kernel(inputw, inputf, weight):
    from concourse import bass_utils

    inputw = np.asarray(inputw, np.float32)
    inputf = np.asarray(inputf, np.float32)
    weight = np.asarray(weight, np.float32)

    if "nc" not in _cache:
        _cache["nc"] = _build_program()
    nc = _cache["nc"]

    # weight layouts (replicated across cores)
    p = np.arange(128)
    wkm = np.empty((128, 8, 9, 64), np.float16)
    for t in range(8):
        iw = 8 * t + p // 16
        jw = p % 16
        wkm[:, t, :, :] = weight[:, :, iw, jw].transpose(2, 0, 1)
    wkm = wkm.reshape(128, 8 * 9 * 64)
    wkx = np.zeros((81, 9, 64), np.float16)
    wkx[:64] = weight[:, :, :64, 16].transpose(2, 0, 1)
    wkx[64:80] = weight[:, :, 64, :16].transpose(2, 0, 1)
    wkx[80] = weight[:, :, 64, 16]
    wkx = wkx.reshape(81, 9 * 64)

    in_maps = []
    for core in range(8):
        b, h = divmod(core, 2)
        fw, frep, wt = _prep_core(inputf, inputw, b, h)
        in_maps.append({"fw": fw, "frep": frep, "wt": wt, "wkm": wkm, "wkx": wkx})

    res = bass_utils.run_bass_kernel_spmd(nc, in_maps, core_ids=list(range(8)))
    kernel.last_result = res

    out = np.empty((B, OCH, HO, WO), np.float32)
    for core in range(8):
        b, h = divmod(core, 2)
        out[b, :, 63 * h:63 * h + 63, :] = res.results[core]["out"][:, :, 0:WO]
    return out

